# revision 1
# baseline (speedup 1.0000x reference)
"""CFD-GCN Trainium2 kernel: 6-layer GCN + KNN-interpolate on 8 NeuronCores.

Strategy (node sharding, feature-major residency):
  - Fine nodes are sharded 6250/core (padded to 6272 = 49*128).
  - Each GCN layer l: z = h @ W_l (dense, per-core shard, PE),
    AllGather(z) -> z_full, per-edge gather of source rows (indirect DMA),
    scatter-add via one-hot matmuls into PSUM per 128-dest tile,
    bias + (relu) on ScalarE -> next h (kept feature-major in SBUF,
    which makes the next dense matmul transpose-free).
  - First layer runs sparse-first on the 6-wide input ((A h0) W0), last
    layer dense-first on the 3-wide output (A (h W5)), so only 4 of the 6
    layers move 512-wide data through the AllGather+gather path.
  - Edges are sorted by destination on the host; each 128-dest tile gets
    a fixed number KB of 128-edge blocks (padded with zero-norm slots).
    The one-hot S[e, d] = norm_e * (col_e == d) is built on DVE from a
    static iota matrix, so scatter-add = sum_j one-hot matmuls
    accumulated in PSUM.
  - KNN-interpolate: -d2 = 2 f.c - |c|^2 - |f|^2 via a K=4 matmul against
    all 2000 coarse nodes, DVE max8/max_index for the top-3, inverse-d2
    weights, tiny indirect gather of coarse_y rows.
"""

import math
import numpy as np

# ---------------------------------------------------------------- constants
N_FINE = 50000
N_COARSE = 2000
HID = 512
OUT = 3
NCORES = 8
P = 128

_PROGRAM_CACHE = {}


# ---------------------------------------------------------------- host side
def _wrap16(flat, P=128):
    """int16 index list -> dma_gather layout [P, len/16] (wrapped in 16
    partitions, replicated across the 8 Q7 cores)."""
    L = len(flat) // 16
    w = np.asarray(flat, np.int16).reshape(L, 16).T  # [16, L]
    return np.tile(w, (P // 16, 1))


def _preprocess_edges(edge_index, n_fine, ncores):
    """Sort edges by destination, shard by dest core, tile dests by 128.

    Edges of each (core, dest-tile) are split by source half (table rows
    0..HALF-1 vs HALF..), each half padded to KBA/KBB 128-edge blocks.

    Returns (KBA, KBB, nt, padsh, per-core dict arrays).
    """
    nsh = n_fine // ncores
    nt = math.ceil(nsh / P)
    padsh = nt * P
    half = (ncores // 2) * padsh

    row = np.asarray(edge_index[0]).astype(np.int64)
    col = np.asarray(edge_index[1]).astype(np.int64)
    loop = np.arange(n_fine, dtype=np.int64)
    row = np.concatenate([row, loop])
    col = np.concatenate([col, loop])

    deg = np.bincount(col, minlength=n_fine).astype(np.float32)
    dis = 1.0 / np.sqrt(deg)
    normv = (dis[row] * dis[col]).astype(np.float32)

    srcpad = ((row // nsh) * padsh + (row % nsh)).astype(np.int64)

    order = np.argsort(col, kind="stable")
    col_s, norm_s, srcpad_s = col[order], normv[order], srcpad[order]

    tiles = []
    KBA = KBB = 1
    for c in range(ncores):
        base = c * nsh
        for t in range(nt):
            lo, hi = base + t * P, min(base + (t + 1) * P, base + nsh)
            a = np.searchsorted(col_s, lo, "left")
            b = np.searchsorted(col_s, hi, "left")
            isa = srcpad_s[a:b] < half
            na, nb = int(isa.sum()), int((~isa).sum())
            tiles.append((c, t, a, b, isa, na, nb))
            KBA = max(KBA, math.ceil(max(na, 1) / P))
            KBB = max(KBB, math.ceil(max(nb, 1) / P))

    KBT = KBA + KBB
    out = []
    for c in range(ncores):
        out.append({
            "idxA": np.full((P, nt * KBA * 8), -1, np.int16),
            "idxB": np.full((P, nt * KBB * 8), -1, np.int16),
            "cntAB": np.ones((2, nt), np.int32),
            "ecol": np.zeros((P, nt * KBT), np.float32),
            "enorm": np.zeros((P, nt * KBT), np.float32),
        })
    for c, t, a, b, isa, na, nb in tiles:
        oc = out[c]
        colrel = (col_s[a:b] - (c * nsh + t * P)).astype(np.float32)
        nrm = norm_s[a:b]
        sp = srcpad_s[a:b]
        for half_i, mask, KBh, key, boff in (
                (0, isa, KBA, "idxA", 0), (1, ~isa, KBB, "idxB", KBA)):
            nh = int(mask.sum())
            flat = np.full(KBh * P, -1, np.int64)
            flat[:nh] = sp[mask] - (half if half_i else 0)
            if nh == 0:
                flat[0] = 0  # dummy valid row, zero norm
            oc[key][:, t * KBh * 8:(t + 1) * KBh * 8] = _wrap16(flat)
            oc["cntAB"][half_i, t] = max(nh, 1)
            s = np.arange(nh)
            oc["ecol"][s % P, t * KBT + boff + s // P] = colrel[mask]
            oc["enorm"][s % P, t * KBT + boff + s // P] = nrm[mask]
    return KBA, KBB, nt, padsh, out


def _pad_shard(x, nsh, padsh, ncores):
    d = x.shape[1]
    out = np.zeros((ncores * padsh, d), x.dtype)
    for c in range(ncores):
        out[c * padsh : c * padsh + nsh] = x[c * nsh : (c + 1) * nsh]
    return out


# ---------------------------------------------------------------- device side
def build_program(n_fine, n_coarse, hid, out_dim, ncores, KBA, KBB, nt):
    import concourse.bass as bass
    import concourse.mybir as mybir
    from concourse.bacc import Bacc
    from concourse.tile import TileContext
    from concourse.masks import make_identity
    from contextlib import ExitStack

    F32 = mybir.dt.float32
    BF16 = mybir.dt.bfloat16
    I32 = mybir.dt.int32
    padsh = nt * P
    npad = ncores * padsh
    half = (ncores // 2) * padsh
    kc = hid // P
    KBT = KBA + KBB
    nblk = nt * KBT
    rg = [list(range(ncores))]
    AF = mybir.ActivationFunctionType
    ALU = mybir.AluOpType
    IOO = bass.IndirectOffsetOnAxis
    ncpad = math.ceil(n_coarse / 512) * 512
    ncc = math.ceil(n_coarse / 512)

    nc = Bacc(num_devices=ncores)

    # ---- kernel I/O (per core) ----
    I16 = mybir.dt.int16
    h0 = nc.declare_dram_parameter("h0", [npad, 64], F32, isOutput=False)
    idxA = nc.declare_dram_parameter("idxA", [P, nt * KBA * 8], I16, isOutput=False)
    idxB = nc.declare_dram_parameter("idxB", [P, nt * KBB * 8], I16, isOutput=False)
    cntAB = nc.declare_dram_parameter("cntAB", [2, nt], I32, isOutput=False)
    ecol = nc.declare_dram_parameter("ecol", [P, nblk], F32, isOutput=False)
    enorm = nc.declare_dram_parameter("enorm", [P, nblk], F32, isOutput=False)
    xposT = nc.declare_dram_parameter("xposT", [2, padsh], F32, isOutput=False)
    xpos_nm = nc.declare_dram_parameter("xpos_nm", [padsh, 2], F32, isOutput=False)
    coarseT = nc.declare_dram_parameter("coarseT", [2, n_coarse], F32, isOutput=False)
    ycoarse = nc.declare_dram_parameter("ycoarse", [n_coarse, out_dim], F32, isOutput=False)
    w_mid = [nc.declare_dram_parameter(n, [hid, hid], F32, isOutput=False)
             for n in ("w1", "w2", "we0", "we1")]
    b_mid = [nc.declare_dram_parameter(n, [hid], F32, isOutput=False)
             for n in ("b1", "b2", "be0", "be1")]
    w0 = nc.declare_dram_parameter("w0", [6, hid], F32, isOutput=False)
    b0 = nc.declare_dram_parameter("b0", [hid], F32, isOutput=False)
    wtop = nc.declare_dram_parameter("wtop", [out_dim, hid], F32, isOutput=False)
    w5 = nc.declare_dram_parameter("w5", [hid, out_dim], F32, isOutput=False)
    b5 = nc.declare_dram_parameter("b5", [out_dim], F32, isOutput=False)
    y_out = nc.declare_dram_parameter("out", [padsh, out_dim], F32, isOutput=True)

    # ---- internal DRAM ----
    zsh = [nc.dram_tensor(f"zsh{i}", [padsh, hid], F32) for i in range(4)]
    zfull = [nc.dram_tensor(f"zfull{i}", [npad, hid], F32, addr_space="Shared")
             for i in range(4)]
    z5sh = nc.dram_tensor("z5sh", [padsh, 64], F32)
    z5full = nc.dram_tensor("z5full", [npad, 64], F32, addr_space="Shared")

    with TileContext(nc) as tc:
        with ExitStack() as ctx:
            main = ctx.enter_context(tc.tile_pool(name="main", bufs=1))
            wpool = ctx.enter_context(tc.tile_pool(name="wpool", bufs=2))
            sp = ctx.enter_context(tc.tile_pool(name="sp", bufs=KBT + 2))
            zp = ctx.enter_context(tc.tile_pool(name="zp", bufs=2))
            smallp = ctx.enter_context(tc.tile_pool(name="smallp", bufs=2))
            # PSUM: three pools, one shared tag each -> 6 banks max
            ppA = ctx.enter_context(tc.tile_pool(name="ppA", bufs=2, space="PSUM"))
            ppB = ctx.enter_context(tc.tile_pool(name="ppB", bufs=2, space="PSUM"))
            ppC = ctx.enter_context(tc.tile_pool(name="ppC", bufs=2, space="PSUM"))

            def accps(shape):
                return ppA.tile(shape, F32, tag="acc", name="acc")

            def densps(shape):
                return ppB.tile(shape, F32, tag="dacc", name="dacc")

            def tps(shape):
                return ppC.tile(shape, F32, tag="tp", name="tp")

            # ---------- persistent tiles ----------
            hT = main.tile([P, kc, padsh], BF16, tag="hT")
            y3n = main.tile([P, nt, out_dim], F32, tag="y3n")  # node-major top3 result
            iota_f = main.tile([P, P], F32, tag="iota_f")
            iden = main.tile([P, P], F32, tag="iden")
            idxA_sb = main.tile([P, nt * KBA * 8], I16, tag="idxA_sb")
            idxB_sb = main.tile([P, nt * KBB * 8], I16, tag="idxB_sb")
            cnt_sb = main.tile([2, nt], I32, tag="cnt_sb")
            ecol_sb = main.tile([P, nblk], F32, tag="ecol_sb")
            enorm_sb = main.tile([P, nblk], F32, tag="enorm_sb")
            wtop_sb = main.tile([out_dim, hid], F32, tag="wtop_sb")

            nc.sync.dma_start(out=idxA_sb[:], in_=idxA[:, :])
            nc.sync.dma_start(out=idxB_sb[:], in_=idxB[:, :])
            nc.sync.dma_start(out=cnt_sb[:], in_=cntAB[:, :])
            nc.sync.dma_start(out=ecol_sb[:], in_=ecol[:, :])
            nc.sync.dma_start(out=enorm_sb[:], in_=enorm[:, :])
            nc.sync.dma_start(out=wtop_sb[:], in_=wtop[:, :])

            iota_i = smallp.tile([P, P], I32, tag="iota_i")
            nc.gpsimd.iota(out=iota_i[:], pattern=[[1, P]], base=0, channel_multiplier=0)
            nc.vector.tensor_copy(out=iota_f[:], in_=iota_i[:])
            make_identity(nc, iden[:])

            # ---------- helpers ----------
            def load_w_mid(wd):
                # SWDGE casts f32 -> bf16 during the DMA
                w_sb = wpool.tile([P, kc, hid], BF16, tag="w_sb")
                nc.gpsimd.dma_start(
                    out=w_sb[:], in_=wd[:, :].rearrange("(k p) h -> p k h", p=P))
                return w_sb

            def load_b_mid(bd):
                b_sb = wpool.tile([P, kc], F32, tag="b_sb")
                nc.sync.dma_start(out=b_sb[:], in_=bd[:].rearrange("(k p) -> p k", p=P))
                return b_sb

            _regctr = [0]

            def edge_gather(msg, t, tableA, tableB, elem):
                # half A -> blocks [0, KBA), half B -> blocks [KBA, KBT)
                for (tab, idx_sb, KBh, boff, hrow) in (
                        (tableA, idxA_sb, KBA, 0, 0),
                        (tableB, idxB_sb, KBB, KBA, 1)):
                    _regctr[0] += 1
                    r = nc.gpsimd.alloc_register(f"gcnt{_regctr[0]}")
                    nc.gpsimd.reg_load(r, cnt_sb[hrow:hrow + 1, t:t + 1])
                    nc.gpsimd.dma_gather(
                        msg[:, boff:boff + KBh, :], tab,
                        idx_sb[:, t * KBh * 8:(t + 1) * KBh * 8],
                        KBh * P, r, elem)

            # persistent ping-pong gather buffers (fixed addresses, zeroed
            # once: -1-skipped slots must read as finite; their norms are 0)
            msg_pp = [main.tile([P, KBT, hid], F32, tag=f"msgpp{i}", name="msgpp")
                      for i in range(2)]
            msg6_pp = [main.tile([P, KBT, 64], F32, tag=f"msg6pp{i}", name="msg6pp")
                       for i in range(2)]
            z5w_pp = [main.tile([P, 64], F32, tag=f"z5wpp{i}", name="z5wpp")
                      for i in range(2)]
            for m in (*msg_pp, *msg6_pp, *z5w_pp):
                nc.gpsimd.memset(m[:], 0.0)

            def make_S(g):
                S = sp.tile([P, P], F32, tag="S")
                nc.vector.tensor_scalar(
                    out=S[:], in0=iota_f[:],
                    scalar1=ecol_sb[:, g:g + 1], scalar2=enorm_sb[:, g:g + 1],
                    op0=ALU.is_equal, op1=ALU.mult)
                return S

            # ---------- KNN (independent; writes y3n) ----------
            # -d2[m, n] = 2 f_m . c_n - |c_n|^2 - |f_m|^2 :
            #   matmul K=3 with lhsT rows [2fx, 2fy, -1], rhs rows
            #   [cx, cy, |c|^2], then a per-partition add of -|f_m|^2.
            with tc.tile_pool(name="knn", bufs=2) as kp:
                mones_sb = kp.tile([1, P], F32, tag="mones_sb", bufs=1)
                nc.vector.memset(mones_sb[:], -1.0)
                coarse3 = kp.tile([3, n_coarse], F32, tag="coarse3", bufs=1)
                with tc.tile_pool(name="knnprep", bufs=1) as kprep:
                    nc.sync.dma_start(out=coarse3[0:2, :], in_=coarseT[:, :])
                    pones = kprep.tile([2, 1], F32, tag="pones")
                    nc.vector.memset(pones[:], 1.0)
                    csq = kprep.tile([1, n_coarse], F32, tag="csq")
                    for i in range(ncc):
                        a, b = i * 512, min((i + 1) * 512, n_coarse)
                        sqc = kprep.tile([2, 512], F32, tag="sqc")
                        nc.vector.tensor_tensor(out=sqc[:, : b - a],
                                                in0=coarse3[0:2, a:b],
                                                in1=coarse3[0:2, a:b], op=ALU.mult)
                        ps = tps([P, 512])
                        nc.tensor.matmul(out=ps[0:1, : b - a], lhsT=pones[:],
                                         rhs=sqc[:, : b - a], start=True, stop=True)
                        nc.vector.tensor_copy(out=csq[:, a:b], in_=ps[0:1, : b - a])
                    # row 2 (|c|^2) via DMA (compute engines can't start at
                    # partition 2)
                    nc.sync.dma_start(out=coarse3[2:3, :], in_=csq[:])

                    # -|f|^2 per node, node-major: [P, nt]
                    xnm = kprep.tile([P, nt, 2], F32, tag="xnm")
                    nc.sync.dma_start(
                        out=xnm[:], in_=xpos_nm[:, :].rearrange("(t p) d -> p t d", p=P))
                    sqn = kprep.tile([P, nt, 2], F32, tag="sqn")
                    nc.vector.tensor_tensor(out=sqn[:], in0=xnm[:], in1=xnm[:],
                                            op=ALU.mult)
                    fsqneg = kp.tile([P, nt], F32, tag="fsqneg", bufs=1)
                    nc.vector.tensor_reduce(out=fsqneg[:], in_=sqn[:],
                                            axis=mybir.AxisListType.X, op=ALU.add,
                                            negate=True)

                for t in range(nt):
                    tp = t * P
                    xp_t = kp.tile([2, P], F32, tag="xp_t")
                    nc.sync.dma_start(out=xp_t[:], in_=xposT[:, tp:tp + P])
                    lhsT3 = kp.tile([3, P], F32, tag="lhsT3")
                    nc.vector.tensor_scalar_mul(lhsT3[0:2, :], xp_t[:], 2.0)
                    nc.sync.dma_start(out=lhsT3[2:3, :], in_=mones_sb[:])

                    d2 = kp.tile([P, ncpad], F32, tag="d2", bufs=1)
                    for i in range(ncc):
                        a, b = i * 512, min((i + 1) * 512, n_coarse)
                        dps = densps([P, 512])
                        nc.tensor.matmul(out=dps[:, : b - a], lhsT=lhsT3[:],
                                         rhs=coarse3[:, a:b], start=True, stop=True)
                        nc.vector.tensor_scalar(out=d2[:, a:b], in0=dps[:, : b - a],
                                                scalar1=fsqneg[:, t:t + 1],
                                                scalar2=None, op0=ALU.add)
                    vals = kp.tile([P, 8], F32, tag="vals")
                    nc.vector.max(out=vals[:], in_=d2[:, 0:n_coarse])
                    idxs = kp.tile([P, 8], mybir.dt.uint32, tag="idxs")
                    nc.vector.max_index(out=idxs[:], in_max=vals[:],
                                        in_values=d2[:, 0:n_coarse])
                    wv = kp.tile([P, 3], F32, tag="wv")
                    nc.vector.tensor_scalar(out=wv[:], in0=vals[:, 0:3],
                                            scalar1=-1.0, scalar2=1e-16,
                                            op0=ALU.mult, op1=ALU.max)
                    nc.vector.reciprocal(out=wv[:], in_=wv[:])
                    wsum = kp.tile([P, 1], F32, tag="wsum")
                    nc.vector.tensor_reduce(out=wsum[:], in_=wv[:],
                                            axis=mybir.AxisListType.X, op=ALU.add)
                    nc.vector.reciprocal(out=wsum[:], in_=wsum[:])
                    nc.vector.tensor_scalar(out=wv[:], in0=wv[:],
                                            scalar1=wsum[:, 0:1], scalar2=None,
                                            op0=ALU.mult)
                    yg = kp.tile([P, 3, out_dim], F32, tag="yg")
                    for k3 in range(3):
                        nc.gpsimd.indirect_dma_start(
                            out=yg[:, k3, :], out_offset=None, in_=ycoarse[:, :],
                            in_offset=IOO(ap=idxs[:, k3:k3 + 1], axis=0))
                    tmp = kp.tile([P, out_dim], F32, tag="tmp")
                    nc.vector.tensor_scalar(out=y3n[:, t, :], in0=yg[:, 0, :],
                                            scalar1=wv[:, 0:1], scalar2=None,
                                            op0=ALU.mult)
                    for k in (1, 2):
                        nc.vector.tensor_scalar(out=tmp[:], in0=yg[:, k, :],
                                                scalar1=wv[:, k:k + 1], scalar2=None,
                                                op0=ALU.mult)
                        nc.vector.tensor_tensor(out=y3n[:, t, :], in0=y3n[:, t, :],
                                                in1=tmp[:], op=ALU.add)

            # ---------- pre0: q = A h0 (6-wide), then z0T = W0^T q, relu ----------
            w0_sb = main.tile([6, hid], F32, tag="w0_sb")
            nc.sync.dma_start(out=w0_sb[:], in_=w0[:, :])
            b0_sb = load_b_mid(b0)
            for t in range(nt):
                tp = t * P
                msg6 = msg6_pp[t % 2]
                edge_gather(msg6, t, h0[0:half, :], h0[half:, :], 64)
                q = accps([P, max(P, kc * P)])
                for j in range(KBT):
                    S = make_S(t * KBT + j)
                    nc.tensor.matmul(out=q[0:6, 0:P], lhsT=msg6[:, j, 0:6], rhs=S[:],
                                     start=(j == 0), stop=(j == KBT - 1))
                q_sb = smallp.tile([6, P], F32, tag="q_sb")
                nc.vector.tensor_copy(out=q_sb[:], in_=q[0:6, 0:P])
                for jj in range(kc):
                    z0 = densps([P, hid])
                    nc.tensor.matmul(out=z0[:, 0:P], lhsT=w0_sb[:, jj * P:(jj + 1) * P],
                                     rhs=q_sb[:], start=True, stop=True)
                    nc.scalar.activation(out=hT[:, jj, tp:tp + P], in_=z0[:, 0:P],
                                         func=AF.Relu, bias=b0_sb[:, jj:jj + 1])

            # ---------- middle layers ----------
            def dense_mid(w_sb, zsh_d, li):
                for t in range(nt):
                    tp = t * P
                    zps = densps([P, hid])
                    for k in range(kc):
                        nc.tensor.matmul(out=zps[:], lhsT=hT[:, k, tp:tp + P],
                                         rhs=w_sb[:, k, :], start=(k == 0),
                                         stop=(k == kc - 1) and li != 2)
                    if li == 2:
                        pt3 = tps([P, P])
                        nc.tensor.transpose(out=pt3[0:out_dim, 0:P],
                                            in_=y3n[:, t, :], identity=iden[:])
                        y3t_T = smallp.tile([out_dim, P], F32, tag="y3t_T")
                        nc.vector.tensor_copy(out=y3t_T[:], in_=pt3[0:out_dim, 0:P])
                        nc.tensor.matmul(out=zps[:], lhsT=y3t_T[:],
                                         rhs=wtop_sb[:, :], start=False, stop=True)
                    z_sb = zp.tile([P, hid], F32, tag="z_sb")
                    nc.scalar.activation(out=z_sb[:], in_=zps[:], func=AF.Copy)
                    nc.sync.dma_start(out=zsh_d[tp:tp + P, :], in_=z_sb[:])

            def sparse_mid(zfull_d, b_sb):
                for t in range(nt):
                    tp = t * P
                    msg = msg_pp[t % 2]
                    edge_gather(msg, t, zfull_d[0:half, :], zfull_d[half:, :], hid)
                    hps = accps([P, kc * P])
                    S_list = [make_S(t * KBT + j) for j in range(KBT)]
                    for cc in range(kc):
                        for j in range(KBT):
                            nc.tensor.matmul(out=hps[:, cc * P:(cc + 1) * P],
                                             lhsT=msg[:, j, cc * P:(cc + 1) * P],
                                             rhs=S_list[j][:], start=(j == 0),
                                             stop=(j == KBT - 1))
                    for cc in range(kc):
                        nc.scalar.activation(out=hT[:, cc, tp:tp + P],
                                             in_=hps[:, cc * P:(cc + 1) * P],
                                             func=AF.Relu, bias=b_sb[:, cc:cc + 1])

            for li in range(4):
                w_sb = load_w_mid(w_mid[li])
                b_sb = load_b_mid(b_mid[li])
                dense_mid(w_sb, zsh[li], li)
                nc.gpsimd.collective_compute(
                    "AllGather", ALU.bypass, replica_groups=rg,
                    ins=[zsh[li][:, :]], outs=[zfull[li][:, :]])
                sparse_mid(zfull[li], b_sb)

            # ---------- end2: z5T = W5^T h, transpose, AG, sparse3 + bias ----------
            w5_sb = main.tile([P, kc, out_dim], BF16, tag="w5_sb")
            nc.gpsimd.dma_start(out=w5_sb[:],
                                in_=w5[:, :].rearrange("(k p) o -> p k o", p=P))
            b5_sb = main.tile([out_dim, 1], F32, tag="b5_sb")
            nc.sync.dma_start(out=b5_sb[:], in_=b5[:, None])

            for t in range(nt):
                tp = t * P
                z5ps = densps([P, hid])
                for k in range(kc):
                    nc.tensor.matmul(out=z5ps[0:out_dim, 0:P], lhsT=w5_sb[:, k, :],
                                     rhs=hT[:, k, tp:tp + P], start=(k == 0),
                                     stop=(k == kc - 1))
                # transpose [3, 128] -> [128, 3] and store node-major
                z5T_sb = smallp.tile([out_dim, P], F32, tag="z5T_sb")
                nc.vector.tensor_copy(out=z5T_sb[:], in_=z5ps[0:out_dim, 0:P])
                ptp = tps([P, P])
                nc.tensor.transpose(out=ptp[:, 0:out_dim], in_=z5T_sb[:],
                                    identity=iden[0:out_dim, 0:out_dim])
                z5_sb = z5w_pp[t % 2]
                nc.vector.tensor_copy(out=z5_sb[:, 0:out_dim], in_=ptp[:, 0:out_dim])
                nc.sync.dma_start(out=z5sh[tp:tp + P, :], in_=z5_sb[:])
            nc.gpsimd.collective_compute(
                "AllGather", ALU.bypass, replica_groups=rg,
                ins=[z5sh[:, :]], outs=[z5full[:, :]])
            for t in range(nt):
                tp = t * P
                msg3 = msg6_pp[t % 2]
                edge_gather(msg3, t, z5full[0:half, :], z5full[half:, :], 64)
                ops = accps([P, max(P, kc * P)])
                for j in range(KBT):
                    S = make_S(t * KBT + j)
                    nc.tensor.matmul(out=ops[0:out_dim, 0:P],
                                     lhsT=msg3[:, j, 0:out_dim],
                                     rhs=S[:], start=(j == 0), stop=(j == KBT - 1))
                oT = smallp.tile([out_dim, P], F32, tag="oT")
                nc.vector.tensor_scalar(out=oT[:], in0=ops[0:out_dim, 0:P],
                                        scalar1=b5_sb[:, 0:1], scalar2=None,
                                        op0=ALU.add)
                po = tps([P, P])
                nc.tensor.transpose(out=po[:, 0:out_dim], in_=oT[:],
                                    identity=iden[0:out_dim, 0:out_dim])
                o_sb = smallp.tile([P, out_dim], F32, tag="o_sb")
                nc.vector.tensor_copy(out=o_sb[:], in_=po[:, 0:out_dim])
                nc.sync.dma_start(out=y_out[tp:tp + P, :], in_=o_sb[:])

    nc.finalize()
    return nc


# ---------------------------------------------------------------- entry point
def _prepare(inputs, n_fine, n_coarse, hid, out_dim, ncores):
    x = np.asarray(inputs["x"], np.float32)
    sdf = np.asarray(inputs["sdf"], np.float32)
    coarse_x = np.asarray(inputs["coarse_x"], np.float32)
    coarse_y = np.asarray(inputs["coarse_y"], np.float32)
    edge_index = np.asarray(inputs["edge_index"])

    KBA, KBB, nt, padsh, edges = _preprocess_edges(edge_index, n_fine, ncores)
    nsh = n_fine // ncores

    h0 = np.zeros((n_fine, 64), np.float32)
    h0[:, 0:5] = x
    h0[:, 5:6] = sdf
    h0pad = _pad_shard(h0, nsh, padsh, ncores)

    xpos = x[:, :2].astype(np.float32)
    xposT = []
    xpos_nm_l = []
    for c in range(ncores):
        xx = np.zeros((2, padsh), np.float32)
        xx[:, :nsh] = xpos[c * nsh:(c + 1) * nsh].T
        xposT.append(xx)
        xn = np.zeros((padsh, 2), np.float32)
        xn[:nsh] = xpos[c * nsh:(c + 1) * nsh]
        xpos_nm_l.append(xn)
    coarseT = np.ascontiguousarray(coarse_x[:, :2].T).astype(np.float32)

    in_maps = []
    for c in range(ncores):
        m = {
            "h0": h0pad,
            "idxA": edges[c]["idxA"], "idxB": edges[c]["idxB"],
            "cntAB": edges[c]["cntAB"],
            "ecol": edges[c]["ecol"], "enorm": edges[c]["enorm"],
            "xposT": xposT[c], "xpos_nm": xpos_nm_l[c],
            "coarseT": coarseT, "ycoarse": coarse_y,
            "w0": np.asarray(inputs["pre_W0"], np.float32),
            "b0": np.asarray(inputs["pre_b0"], np.float32),
            "w1": np.asarray(inputs["pre_W1"], np.float32),
            "b1": np.asarray(inputs["pre_b1"], np.float32),
            "w2": np.asarray(inputs["pre_W2"], np.float32),
            "b2": np.asarray(inputs["pre_b2"], np.float32),
            # end_W0 is [OUT+HID, HID]: top 3 rows couple y3, rest couple h
            "wtop": np.ascontiguousarray(np.asarray(inputs["end_W0"], np.float32)[:out_dim]),
            "we0": np.ascontiguousarray(np.asarray(inputs["end_W0"], np.float32)[out_dim:]),
            "be0": np.asarray(inputs["end_b0"], np.float32),
            "we1": np.asarray(inputs["end_W1"], np.float32),
            "be1": np.asarray(inputs["end_b1"], np.float32),
            "w5": np.asarray(inputs["end_W2"], np.float32),
            "b5": np.asarray(inputs["end_b2"], np.float32),
        }
        in_maps.append(m)
    return KBA, KBB, nt, padsh, in_maps


def run(inputs, n_fine=N_FINE, n_coarse=N_COARSE, hid=HID, out_dim=OUT,
        ncores=NCORES, sim=False, trace=False):
    KBA, KBB, nt, padsh, in_maps = _prepare(inputs, n_fine, n_coarse, hid,
                                            out_dim, ncores)
    key = (n_fine, n_coarse, hid, out_dim, ncores, KBA, KBB, nt)
    if key not in _PROGRAM_CACHE:
        _PROGRAM_CACHE[key] = build_program(n_fine, n_coarse, hid, out_dim,
                                            ncores, KBA, KBB, nt)
    nc = _PROGRAM_CACHE[key]

    nsh = n_fine // ncores
    if sim:
        from concourse.bass_interp import MultiCoreSim
        ms = MultiCoreSim(nc, ncores, num_workers=1)
        for c in range(ncores):
            for k, v in in_maps[c].items():
                ms.cores[c].tensor(k)[:] = v
        ms.simulate()
        outs = [np.array(ms.cores[c].tensor("out")) for c in range(ncores)]
        exec_ns = None
    else:
        from concourse.bass_utils import run_bass_kernel_spmd
        res = run_bass_kernel_spmd(nc, in_maps, list(range(ncores)), trace=trace)
        outs = [res.results[c]["out"] for c in range(ncores)]
        exec_ns = res.exec_time_ns

    full = np.zeros((n_fine, out_dim), np.float32)
    for c in range(ncores):
        full[c * nsh:(c + 1) * nsh] = outs[c][:nsh]
    return full, exec_ns


def kernel(**inputs):
    out, _ = run(inputs)
    return out



# revision 8
# speedup vs baseline: 1.2257x; 1.2257x over previous
"""CFD-GCN Trainium2 kernel: 6-layer GCN + KNN-interpolate on 8 NeuronCores.

Strategy (node sharding, feature-major residency, v2):
  - Fine nodes are sharded 6250/core (padded to 6272 = 49*128).
  - Each GCN layer l: z = h @ W_l (dense, per-core shard, PE),
    AllGather(z) -> z_full (fp16), per-edge gather of source rows
    (grouped indirect DMA over ~3-4 dest tiles per call), scatter-add
    via one-hot matmuls into PSUM per 128-dest tile, bias + (relu) on
    ScalarE -> next h (kept feature-major in SBUF, which makes the next
    dense matmul transpose-free).
  - First layer runs sparse-first on the 6-wide input ((A h0) W0), last
    layer dense-first on the 3-wide output (A (h W5)), so only 4 of the
    6 layers move 512-wide data through the AllGather+gather path.
  - The one-hot scatter matrices S[e, d] = norm_e * (col_e == d) are
    built on the HOST (fp16) and streamed from DRAM, so DVE does no
    S-building work. Edges are sorted by destination; each 128-dest
    tile gets a data-dependent number of 128-edge blocks (max over
    cores so the program is SPMD-uniform), padded with (idx=0, norm=0)
    slots. Blocks of several tiles are gathered in one dma_gather call.
  - Everything on the wide path (z_full, gathered messages, S, h, W) is
    fp16; PSUM accumulation stays fp32.
  - KNN-interpolate: -d2 = 2 f.c - |c|^2 - |f|^2 via a K=3 matmul
    against all coarse nodes, DVE max8/max_index for the top-3,
    inverse-d2 weights, tiny indirect gather of coarse_y rows.
"""

import math
import numpy as np
import ml_dtypes

HDT = np.float16

# ---------------------------------------------------------------- constants
N_FINE = 50000
N_COARSE = 2000
HID = 512
OUT = 3
NCORES = 8
P = 128
GMAXB = 28  # max edge-blocks (128 edges each) per gather group

_PROGRAM_CACHE = {}


# ---------------------------------------------------------------- host side
def _wrap16(flat, P=128):
    """int16 index list -> dma_gather layout [P, len/16] (wrapped in 16
    partitions, replicated across the 8 Q7 cores)."""
    L = len(flat) // 16
    w = np.asarray(flat, np.int16).reshape(L, 16).T  # [16, L]
    return np.tile(w, (P // 16, 1))


class _Group:
    __slots__ = ("tiles", "blkA", "blkB", "blk", "gb0")


class _Plan:
    __slots__ = ("nt", "padsh", "half", "groups", "totblk", "gmax",
                 "blkA", "blkB", "key")


def _make_plan(cntA, cntB, nt, padsh, half):
    """Uniform (max-over-cores) per-tile block counts and gather groups."""
    blkA = np.ceil(np.maximum(cntA.max(axis=0), 1) / P).astype(int)
    blkB = np.ceil(np.maximum(cntB.max(axis=0), 1) / P).astype(int)

    groups = []
    cur = []
    for t in range(nt):
        bt = int(blkA[t] + blkB[t])
        curblk = sum(int(blkA[u] + blkB[u]) for u in cur)
        if cur and curblk + bt > GMAXB:
            groups.append(list(cur))
            cur = []
        cur.append(t)
    if cur:
        groups.append(cur)

    plan = _Plan()
    plan.nt, plan.padsh, plan.half = nt, padsh, half
    plan.blkA, plan.blkB = blkA, blkB
    plan.groups = []
    gb0 = 0
    for tiles in groups:
        g = _Group()
        g.tiles = []
        aoff = 0
        for t in tiles:
            g.tiles.append((t, aoff, int(blkA[t]), None, int(blkB[t])))
            aoff += int(blkA[t])
        g.blkA = aoff
        boff = 0
        tl = []
        for (t, ao, na, _, nb) in g.tiles:
            tl.append((t, ao, na, boff, nb))
            boff += nb
        g.tiles = tl
        g.blkB = boff
        g.blk = g.blkA + g.blkB
        g.gb0 = gb0
        gb0 += g.blk
        plan.groups.append(g)
    plan.totblk = gb0
    plan.gmax = max(g.blk for g in plan.groups)
    plan.key = (nt, padsh, half, tuple(int(x) for x in blkA),
                tuple(int(x) for x in blkB))
    return plan


def _preprocess_edges(edge_index, n_fine, ncores):
    """Sort edges by destination, shard by dest core, tile dests by 128.

    Edges of each (core, dest-tile) are split by source half (table rows
    0..HALF-1 vs HALF..) so indices fit int16; each (tile, half) segment
    is padded to a multiple of 128 edges with (idx=0, norm=0) slots.

    Returns (plan, per-core dict arrays {idx16, S}).
    """
    nsh = n_fine // ncores
    nt = math.ceil(nsh / P)
    padsh = nt * P
    half = (ncores // 2) * padsh

    row = np.asarray(edge_index[0]).astype(np.int64)
    col = np.asarray(edge_index[1]).astype(np.int64)
    loop = np.arange(n_fine, dtype=np.int64)
    row = np.concatenate([row, loop])
    col = np.concatenate([col, loop])

    deg = np.bincount(col, minlength=n_fine).astype(np.float32)
    dis = 1.0 / np.sqrt(deg)
    normv = (dis[row] * dis[col]).astype(np.float32)
    srcpad = ((row // nsh) * padsh + (row % nsh)).astype(np.int64)

    order = np.argsort(col, kind="stable")
    col_s, norm_s, srcpad_s = col[order], normv[order], srcpad[order]

    seg = {}
    cntA = np.zeros((ncores, nt), np.int64)
    cntB = np.zeros((ncores, nt), np.int64)
    for c in range(ncores):
        base = c * nsh
        for t in range(nt):
            lo, hi = base + t * P, min(base + (t + 1) * P, base + nsh)
            a = np.searchsorted(col_s, lo, "left")
            b = np.searchsorted(col_s, hi, "left")
            isa = srcpad_s[a:b] < half
            seg[(c, t)] = (a, b, isa)
            cntA[c, t] = int(isa.sum())
            cntB[c, t] = int((~isa).sum())

    plan = _make_plan(cntA, cntB, nt, padsh, half)

    # global block base per (tile, half)
    baseA = np.zeros(nt, np.int64)
    baseB = np.zeros(nt, np.int64)
    for g in plan.groups:
        for (t, ao, na, bo, nb) in g.tiles:
            baseA[t] = g.gb0 + ao
            baseB[t] = g.gb0 + g.blkA + bo

    out = []
    for c in range(ncores):
        idx_flat = np.zeros(plan.totblk * P, np.int64)
        pL, bL, cL, nL = [], [], [], []
        for t in range(nt):
            a, b, isa = seg[(c, t)]
            colrel = (col_s[a:b] - (c * nsh + t * P)).astype(np.int64)
            nrm = norm_s[a:b]
            sp = srcpad_s[a:b]
            for (mask, bbase, hoff) in ((isa, baseA[t], 0),
                                        (~isa, baseB[t], half)):
                src = sp[mask] - hoff
                nh = len(src)
                if nh == 0:
                    continue
                s = np.arange(nh)
                idx_flat[bbase * P + s] = src
                pL.append(s % P)
                bL.append(bbase + s // P)
                cL.append(colrel[mask])
                nL.append(nrm[mask])
        S_np = np.zeros((P, plan.totblk, P), np.float32)
        S_np[np.concatenate(pL), np.concatenate(bL),
             np.concatenate(cL)] = np.concatenate(nL)
        out.append({
            "idx16": _wrap16(idx_flat),
            "S": S_np.astype(HDT),
        })
    return plan, out


def _pad_shard(x, nsh, padsh, ncores):
    d = x.shape[1]
    out = np.zeros((ncores * padsh, d), x.dtype)
    for c in range(ncores):
        out[c * padsh: c * padsh + nsh] = x[c * nsh: (c + 1) * nsh]
    return out


# ---------------------------------------------------------------- device side
def build_program(n_fine, n_coarse, hid, out_dim, ncores, plan):
    import concourse.bass as bass
    import concourse.mybir as mybir
    from concourse.bacc import Bacc
    from concourse.tile import TileContext
    from concourse.masks import make_identity
    from contextlib import ExitStack

    F32 = mybir.dt.float32
    F16 = mybir.dt.float16
    I16 = mybir.dt.int16
    nt, padsh, half = plan.nt, plan.padsh, plan.half
    npad = ncores * padsh
    kc = hid // P
    totblk = plan.totblk
    gmax = plan.gmax
    rg = [list(range(ncores))]
    AF = mybir.ActivationFunctionType
    ALU = mybir.AluOpType
    IOO = bass.IndirectOffsetOnAxis
    ncpad = math.ceil(n_coarse / 512) * 512
    ncc = math.ceil(n_coarse / 512)

    nc = Bacc(num_devices=ncores)

    # ---- kernel I/O (per core) ----
    h0 = nc.declare_dram_parameter("h0", [npad, P], F16, isOutput=False)
    idx16 = nc.declare_dram_parameter("idx16", [P, totblk * 8], I16,
                                      isOutput=False)
    S_dram = nc.declare_dram_parameter("S", [P, totblk, P], F16,
                                       isOutput=False)
    xposT = nc.declare_dram_parameter("xposT", [2, padsh], F32, isOutput=False)
    xpos_nm = nc.declare_dram_parameter("xpos_nm", [padsh, 2], F32,
                                        isOutput=False)
    coarseT = nc.declare_dram_parameter("coarseT", [2, n_coarse], F32,
                                        isOutput=False)
    ycoarse = nc.declare_dram_parameter("ycoarse", [n_coarse, out_dim], F32,
                                        isOutput=False)
    w_mid = [nc.declare_dram_parameter(n, [hid, hid], F32, isOutput=False)
             for n in ("w1", "w2", "we0", "we1")]
    b_mid = [nc.declare_dram_parameter(n, [hid], F32, isOutput=False)
             for n in ("b1", "b2", "be0", "be1")]
    w0 = nc.declare_dram_parameter("w0", [6, hid], F32, isOutput=False)
    b0 = nc.declare_dram_parameter("b0", [hid], F32, isOutput=False)
    wtop = nc.declare_dram_parameter("wtop", [out_dim, hid], F32,
                                     isOutput=False)
    w5 = nc.declare_dram_parameter("w5", [hid, out_dim], F32, isOutput=False)
    b5 = nc.declare_dram_parameter("b5", [out_dim], F32, isOutput=False)
    y_out = nc.declare_dram_parameter("out", [padsh, out_dim], F32,
                                      isOutput=True)

    # ---- internal DRAM ----
    zsh = [nc.dram_tensor(f"zsh{i}", [padsh, hid], F16) for i in range(4)]
    zfull = [nc.dram_tensor(f"zfull{i}", [npad, hid], F16, addr_space="Shared")
             for i in range(4)]
    z5sh = nc.dram_tensor("z5sh", [padsh, P], F16)
    z5full = nc.dram_tensor("z5full", [npad, P], F16, addr_space="Shared")

    with TileContext(nc) as tc:
        with ExitStack() as ctx:
            main = ctx.enter_context(tc.tile_pool(name="main", bufs=1))
            wpool = ctx.enter_context(tc.tile_pool(name="wpool", bufs=2))
            sp = ctx.enter_context(tc.tile_pool(name="sp", bufs=2))
            zp = ctx.enter_context(tc.tile_pool(name="zp", bufs=2))
            smallp = ctx.enter_context(tc.tile_pool(name="smallp", bufs=2))
            # PSUM: three pools, one shared tag each -> 6 banks max
            ppA = ctx.enter_context(tc.tile_pool(name="ppA", bufs=2,
                                                 space="PSUM"))
            ppB = ctx.enter_context(tc.tile_pool(name="ppB", bufs=2,
                                                 space="PSUM"))
            ppC = ctx.enter_context(tc.tile_pool(name="ppC", bufs=2,
                                                 space="PSUM"))

            def accps(shape):
                return ppA.tile(shape, F32, tag="acc", name="acc")

            def densps(shape):
                return ppB.tile(shape, F32, tag="dacc", name="dacc")

            def tps(shape):
                return ppC.tile(shape, F32, tag="tp", name="tp")

            # ---------- persistent tiles ----------
            hT = main.tile([P, kc, padsh], F16, tag="hT")
            y3n = main.tile([P, nt, out_dim], F32, tag="y3n")
            iden = main.tile([P, P], F32, tag="iden")
            idx_sb = main.tile([P, totblk * 8], I16, tag="idx_sb")
            wtop_sb = main.tile([out_dim, hid], F16, tag="wtop_sb")
            w0_sb = main.tile([6, hid], F16, tag="w0_sb")

            nc.sync.dma_start(out=idx_sb[:], in_=idx16[:, :])
            nc.gpsimd.dma_start(out=wtop_sb[:], in_=wtop[:, :])
            nc.gpsimd.dma_start(out=w0_sb[:], in_=w0[:, :])
            make_identity(nc, iden[:])

            # ---------- helpers ----------
            def load_w_mid(wd):
                # SWDGE casts f32 -> fp16 during the DMA
                w_sb = wpool.tile([P, kc, hid], F16, tag="w_sb")
                nc.gpsimd.dma_start(
                    out=w_sb[:], in_=wd[:, :].rearrange("(k p) h -> p k h",
                                                        p=P))
                return w_sb

            def load_b_mid(bd):
                b_sb = wpool.tile([P, kc], F32, tag="b_sb")
                nc.sync.dma_start(out=b_sb[:],
                                  in_=bd[:].rearrange("(k p) -> p k", p=P))
                return b_sb

            def sparse_pass(tableA, tableB, elem, msg_tag, per_tile):
                """Grouped gathers + per-tile one-hot matmuls.

                per_tile(t, msg, S_sb, blocks) consumes the gathered
                messages; blocks is the list of local block indices of
                tile t within the group buffers.
                """
                for g in plan.groups:
                    msg = sp.tile([P, gmax, elem], F16, tag=msg_tag,
                                  name=msg_tag)
                    S_sb = sp.tile([P, gmax, P], F16, tag="S_" + msg_tag,
                                   name="S_sb")
                    if g.blkA:
                        nc.gpsimd.dma_gather(
                            msg[:, 0:g.blkA, :], tableA,
                            idx_sb[:, g.gb0 * 8:(g.gb0 + g.blkA) * 8],
                            g.blkA * P, g.blkA * P, elem,
                            single_packet=False)
                    if g.blkB:
                        nc.gpsimd.dma_gather(
                            msg[:, g.blkA:g.blk, :], tableB,
                            idx_sb[:, (g.gb0 + g.blkA) * 8:
                                   (g.gb0 + g.blk) * 8],
                            g.blkB * P, g.blkB * P, elem,
                            single_packet=False)
                    nc.sync.dma_start(out=S_sb[:, 0:g.blk, :],
                                      in_=S_dram[:, g.gb0:g.gb0 + g.blk, :])
                    for (t, ao, na, bo, nb) in g.tiles:
                        blocks = (list(range(ao, ao + na)) +
                                  list(range(g.blkA + bo, g.blkA + bo + nb)))
                        per_tile(t, msg, S_sb, blocks)

            # ---------- KNN (independent; writes y3n) ----------
            # -d2[m, n] = 2 f_m . c_n - |c_n|^2 - |f_m|^2 :
            #   matmul K=3 with lhsT rows [2fx, 2fy, -1], rhs rows
            #   [cx, cy, |c|^2], then a per-partition add of -|f_m|^2.
            with tc.tile_pool(name="knn", bufs=2) as kp:
                mones_sb = kp.tile([1, P], F32, tag="mones_sb", bufs=1)
                nc.vector.memset(mones_sb[:], -1.0)
                coarse3 = kp.tile([3, n_coarse], F32, tag="coarse3", bufs=1)
                with tc.tile_pool(name="knnprep", bufs=1) as kprep:
                    nc.sync.dma_start(out=coarse3[0:2, :], in_=coarseT[:, :])
                    pones = kprep.tile([2, 1], F32, tag="pones")
                    nc.vector.memset(pones[:], 1.0)
                    csq = kprep.tile([1, n_coarse], F32, tag="csq")
                    for i in range(ncc):
                        a, b = i * 512, min((i + 1) * 512, n_coarse)
                        sqc = kprep.tile([2, 512], F32, tag="sqc")
                        nc.vector.tensor_tensor(out=sqc[:, : b - a],
                                                in0=coarse3[0:2, a:b],
                                                in1=coarse3[0:2, a:b],
                                                op=ALU.mult)
                        ps = tps([P, 512])
                        nc.tensor.matmul(out=ps[0:1, : b - a], lhsT=pones[:],
                                         rhs=sqc[:, : b - a], start=True,
                                         stop=True)
                        nc.vector.tensor_copy(out=csq[:, a:b],
                                              in_=ps[0:1, : b - a])
                    # row 2 (|c|^2) via DMA (compute engines can't start at
                    # partition 2)
                    nc.sync.dma_start(out=coarse3[2:3, :], in_=csq[:])

                    # -|f|^2 per node, node-major: [P, nt]
                    xnm = kprep.tile([P, nt, 2], F32, tag="xnm")
                    nc.sync.dma_start(
                        out=xnm[:],
                        in_=xpos_nm[:, :].rearrange("(t p) d -> p t d", p=P))
                    sqn = kprep.tile([P, nt, 2], F32, tag="sqn")
                    nc.vector.tensor_tensor(out=sqn[:], in0=xnm[:], in1=xnm[:],
                                            op=ALU.mult)
                    fsqneg = kp.tile([P, nt], F32, tag="fsqneg", bufs=1)
                    nc.vector.tensor_reduce(out=fsqneg[:], in_=sqn[:],
                                            axis=mybir.AxisListType.X,
                                            op=ALU.add, negate=True)

                for t in range(nt):
                    tp = t * P
                    xp_t = kp.tile([2, P], F32, tag="xp_t")
                    nc.sync.dma_start(out=xp_t[:], in_=xposT[:, tp:tp + P])
                    lhsT3 = kp.tile([3, P], F32, tag="lhsT3")
                    nc.vector.tensor_scalar_mul(lhsT3[0:2, :], xp_t[:], 2.0)
                    nc.sync.dma_start(out=lhsT3[2:3, :], in_=mones_sb[:])

                    d2 = kp.tile([P, ncpad], F32, tag="d2", bufs=1)
                    for i in range(ncc):
                        a, b = i * 512, min((i + 1) * 512, n_coarse)
                        dps = densps([P, 512])
                        nc.tensor.matmul(out=dps[:, : b - a], lhsT=lhsT3[:],
                                         rhs=coarse3[:, a:b], start=True,
                                         stop=True)
                        nc.vector.tensor_scalar(out=d2[:, a:b],
                                                in0=dps[:, : b - a],
                                                scalar1=fsqneg[:, t:t + 1],
                                                scalar2=None, op0=ALU.add)
                    vals = kp.tile([P, 8], F32, tag="vals")
                    nc.vector.max(out=vals[:], in_=d2[:, 0:n_coarse])
                    idxs = kp.tile([P, 8], mybir.dt.uint32, tag="idxs")
                    nc.vector.max_index(out=idxs[:], in_max=vals[:],
                                        in_values=d2[:, 0:n_coarse])
                    wv = kp.tile([P, 3], F32, tag="wv")
                    nc.vector.tensor_scalar(out=wv[:], in0=vals[:, 0:3],
                                            scalar1=-1.0, scalar2=1e-16,
                                            op0=ALU.mult, op1=ALU.max)
                    nc.vector.reciprocal(out=wv[:], in_=wv[:])
                    wsum = kp.tile([P, 1], F32, tag="wsum")
                    nc.vector.tensor_reduce(out=wsum[:], in_=wv[:],
                                            axis=mybir.AxisListType.X,
                                            op=ALU.add)
                    nc.vector.reciprocal(out=wsum[:], in_=wsum[:])
                    nc.vector.tensor_scalar(out=wv[:], in0=wv[:],
                                            scalar1=wsum[:, 0:1], scalar2=None,
                                            op0=ALU.mult)
                    yg = kp.tile([P, 3, out_dim], F32, tag="yg")
                    for k3 in range(3):
                        nc.gpsimd.indirect_dma_start(
                            out=yg[:, k3, :], out_offset=None,
                            in_=ycoarse[:, :],
                            in_offset=IOO(ap=idxs[:, k3:k3 + 1], axis=0))
                    tmp = kp.tile([P, out_dim], F32, tag="tmp")
                    nc.vector.tensor_scalar(out=y3n[:, t, :], in0=yg[:, 0, :],
                                            scalar1=wv[:, 0:1], scalar2=None,
                                            op0=ALU.mult)
                    for k in (1, 2):
                        nc.vector.tensor_scalar(out=tmp[:], in0=yg[:, k, :],
                                                scalar1=wv[:, k:k + 1],
                                                scalar2=None, op0=ALU.mult)
                        nc.vector.tensor_tensor(out=y3n[:, t, :],
                                                in0=y3n[:, t, :],
                                                in1=tmp[:], op=ALU.add)

            # ---------- pre0: q = A h0 (6-wide), then z0T = W0^T q, relu ----
            b0_sb = load_b_mid(b0)

            def pre0_tile(t, msg, S_sb, blocks):
                tp = t * P
                q = accps([P, max(P, kc * P)])
                nbk = len(blocks)
                for i, bj in enumerate(blocks):
                    nc.tensor.matmul(out=q[0:6, 0:P], lhsT=msg[:, bj, 0:6],
                                     rhs=S_sb[:, bj, :], start=(i == 0),
                                     stop=(i == nbk - 1))
                q_sb = smallp.tile([6, P], F16, tag="q_sb")
                nc.vector.tensor_copy(out=q_sb[:], in_=q[0:6, 0:P])
                for jj in range(kc):
                    z0 = densps([P, hid])
                    nc.tensor.matmul(out=z0[:, 0:P],
                                     lhsT=w0_sb[:, jj * P:(jj + 1) * P],
                                     rhs=q_sb[:], start=True, stop=True)
                    nc.scalar.activation(out=hT[:, jj, tp:tp + P],
                                         in_=z0[:, 0:P], func=AF.Relu,
                                         bias=b0_sb[:, jj:jj + 1])

            sparse_pass(h0[0:half, :], h0[half:, :], P, "msgS", pre0_tile)

            # ---------- middle layers ----------
            def dense_mid(w_sb, zsh_d, li):
                for t in range(nt):
                    tp = t * P
                    zps = densps([P, hid])
                    for k in range(kc):
                        nc.tensor.matmul(out=zps[:], lhsT=hT[:, k, tp:tp + P],
                                         rhs=w_sb[:, k, :], start=(k == 0),
                                         stop=(k == kc - 1) and li != 2)
                    if li == 2:
                        pt3 = tps([P, P])
                        nc.tensor.transpose(out=pt3[0:out_dim, 0:P],
                                            in_=y3n[:, t, :], identity=iden[:])
                        y3t_T = smallp.tile([out_dim, P], F16, tag="y3t_T")
                        nc.vector.tensor_copy(out=y3t_T[:],
                                              in_=pt3[0:out_dim, 0:P])
                        nc.tensor.matmul(out=zps[:], lhsT=y3t_T[:],
                                         rhs=wtop_sb[:, :], start=False,
                                         stop=True)
                    z_sb = zp.tile([P, hid], F16, tag="z_sb")
                    nc.scalar.activation(out=z_sb[:], in_=zps[:], func=AF.Copy)
                    nc.sync.dma_start(out=zsh_d[tp:tp + P, :], in_=z_sb[:])

            def mid_tile_fn(b_sb):
                def mid_tile(t, msg, S_sb, blocks):
                    tp = t * P
                    hps = accps([P, kc * P])
                    nbk = len(blocks)
                    for cc in range(kc):
                        for i, bj in enumerate(blocks):
                            nc.tensor.matmul(
                                out=hps[:, cc * P:(cc + 1) * P],
                                lhsT=msg[:, bj, cc * P:(cc + 1) * P],
                                rhs=S_sb[:, bj, :], start=(i == 0),
                                stop=(i == nbk - 1))
                    for cc in range(kc):
                        nc.scalar.activation(out=hT[:, cc, tp:tp + P],
                                             in_=hps[:, cc * P:(cc + 1) * P],
                                             func=AF.Relu,
                                             bias=b_sb[:, cc:cc + 1])
                return mid_tile

            for li in range(4):
                w_sb = load_w_mid(w_mid[li])
                b_sb = load_b_mid(b_mid[li])
                dense_mid(w_sb, zsh[li], li)
                nc.gpsimd.collective_compute(
                    "AllGather", ALU.bypass, replica_groups=rg,
                    ins=[zsh[li][:, :]], outs=[zfull[li][:, :]])
                sparse_pass(zfull[li][0:half, :], zfull[li][half:, :], hid,
                            "msgW", mid_tile_fn(b_sb))

            # ---------- end2: z5T = W5^T h, transpose, AG, sparse3 + bias ----
            w5_sb = main.tile([P, kc, out_dim], F16, tag="w5_sb")
            nc.gpsimd.dma_start(out=w5_sb[:],
                                in_=w5[:, :].rearrange("(k p) o -> p k o",
                                                       p=P))
            b5_sb = main.tile([out_dim, 1], F32, tag="b5_sb")
            nc.sync.dma_start(out=b5_sb[:], in_=b5[:, None])

            z5w_pp = [main.tile([P, P], F16, tag=f"z5wpp{i}", name="z5wpp")
                      for i in range(2)]
            for m in z5w_pp:
                nc.gpsimd.memset(m[:], 0.0)

            for t in range(nt):
                tp = t * P
                z5ps = densps([P, hid])
                for k in range(kc):
                    nc.tensor.matmul(out=z5ps[0:out_dim, 0:P],
                                     lhsT=w5_sb[:, k, :],
                                     rhs=hT[:, k, tp:tp + P], start=(k == 0),
                                     stop=(k == kc - 1))
                # transpose [3, 128] -> [128, 3] and store node-major
                z5T_sb = smallp.tile([out_dim, P], F32, tag="z5T_sb")
                nc.vector.tensor_copy(out=z5T_sb[:], in_=z5ps[0:out_dim, 0:P])
                ptp = tps([P, P])
                nc.tensor.transpose(out=ptp[:, 0:out_dim], in_=z5T_sb[:],
                                    identity=iden[0:out_dim, 0:out_dim])
                z5_sb = z5w_pp[t % 2]
                nc.vector.tensor_copy(out=z5_sb[:, 0:out_dim],
                                      in_=ptp[:, 0:out_dim])
                nc.sync.dma_start(out=z5sh[tp:tp + P, :], in_=z5_sb[:])
            nc.gpsimd.collective_compute(
                "AllGather", ALU.bypass, replica_groups=rg,
                ins=[z5sh[:, :]], outs=[z5full[:, :]])

            def end_tile(t, msg, S_sb, blocks):
                tp = t * P
                ops = accps([P, max(P, kc * P)])
                nbk = len(blocks)
                for i, bj in enumerate(blocks):
                    nc.tensor.matmul(out=ops[0:out_dim, 0:P],
                                     lhsT=msg[:, bj, 0:out_dim],
                                     rhs=S_sb[:, bj, :], start=(i == 0),
                                     stop=(i == nbk - 1))
                oT = smallp.tile([out_dim, P], F32, tag="oT")
                nc.vector.tensor_scalar(out=oT[:], in0=ops[0:out_dim, 0:P],
                                        scalar1=b5_sb[:, 0:1], scalar2=None,
                                        op0=ALU.add)
                po = tps([P, P])
                nc.tensor.transpose(out=po[:, 0:out_dim], in_=oT[:],
                                    identity=iden[0:out_dim, 0:out_dim])
                o_sb = smallp.tile([P, out_dim], F32, tag="o_sb")
                nc.vector.tensor_copy(out=o_sb[:], in_=po[:, 0:out_dim])
                nc.sync.dma_start(out=y_out[tp:tp + P, :], in_=o_sb[:])

            sparse_pass(z5full[0:half, :], z5full[half:, :], P, "msgS",
                        end_tile)

    nc.finalize()
    return nc


# ---------------------------------------------------------------- entry point
def _prepare(inputs, n_fine, n_coarse, hid, out_dim, ncores):
    x = np.asarray(inputs["x"], np.float32)
    sdf = np.asarray(inputs["sdf"], np.float32)
    coarse_x = np.asarray(inputs["coarse_x"], np.float32)
    coarse_y = np.asarray(inputs["coarse_y"], np.float32)
    edge_index = np.asarray(inputs["edge_index"])

    plan, edges = _preprocess_edges(edge_index, n_fine, ncores)
    nsh = n_fine // ncores
    padsh = plan.padsh

    h0 = np.zeros((n_fine, P), np.float32)
    h0[:, 0:5] = x
    h0[:, 5:6] = sdf
    h0pad = _pad_shard(h0, nsh, padsh, ncores).astype(HDT)

    xpos = x[:, :2].astype(np.float32)
    xposT = []
    xpos_nm_l = []
    for c in range(ncores):
        xx = np.zeros((2, padsh), np.float32)
        xx[:, :nsh] = xpos[c * nsh:(c + 1) * nsh].T
        xposT.append(xx)
        xn = np.zeros((padsh, 2), np.float32)
        xn[:nsh] = xpos[c * nsh:(c + 1) * nsh]
        xpos_nm_l.append(xn)
    coarseT = np.ascontiguousarray(coarse_x[:, :2].T).astype(np.float32)

    in_maps = []
    for c in range(ncores):
        m = {
            "h0": h0pad,
            "idx16": edges[c]["idx16"],
            "S": edges[c]["S"],
            "xposT": xposT[c], "xpos_nm": xpos_nm_l[c],
            "coarseT": coarseT, "ycoarse": coarse_y,
            "w0": np.asarray(inputs["pre_W0"], np.float32),
            "b0": np.asarray(inputs["pre_b0"], np.float32),
            "w1": np.asarray(inputs["pre_W1"], np.float32),
            "b1": np.asarray(inputs["pre_b1"], np.float32),
            "w2": np.asarray(inputs["pre_W2"], np.float32),
            "b2": np.asarray(inputs["pre_b2"], np.float32),
            # end_W0 is [OUT+HID, HID]: top 3 rows couple y3, rest couple h
            "wtop": np.ascontiguousarray(
                np.asarray(inputs["end_W0"], np.float32)[:out_dim]),
            "we0": np.ascontiguousarray(
                np.asarray(inputs["end_W0"], np.float32)[out_dim:]),
            "be0": np.asarray(inputs["end_b0"], np.float32),
            "we1": np.asarray(inputs["end_W1"], np.float32),
            "be1": np.asarray(inputs["end_b1"], np.float32),
            "w5": np.asarray(inputs["end_W2"], np.float32),
            "b5": np.asarray(inputs["end_b2"], np.float32),
        }
        in_maps.append(m)
    return plan, in_maps


def run(inputs, n_fine=N_FINE, n_coarse=N_COARSE, hid=HID, out_dim=OUT,
        ncores=NCORES, sim=False, trace=False):
    plan, in_maps = _prepare(inputs, n_fine, n_coarse, hid, out_dim, ncores)
    key = (n_fine, n_coarse, hid, out_dim, ncores) + plan.key
    if key not in _PROGRAM_CACHE:
        _PROGRAM_CACHE[key] = build_program(n_fine, n_coarse, hid, out_dim,
                                            ncores, plan)
    nc = _PROGRAM_CACHE[key]

    nsh = n_fine // ncores
    if sim:
        from concourse.bass_interp import MultiCoreSim
        ms = MultiCoreSim(nc, ncores, num_workers=1)
        for c in range(ncores):
            for k, v in in_maps[c].items():
                ms.cores[c].tensor(k)[:] = v
        ms.simulate()
        outs = [np.array(ms.cores[c].tensor("out")) for c in range(ncores)]
        exec_ns = None
    else:
        from concourse.bass_utils import run_bass_kernel_spmd
        res = run_bass_kernel_spmd(nc, in_maps, list(range(ncores)),
                                   trace=trace)
        outs = [res.results[c]["out"] for c in range(ncores)]
        exec_ns = res.exec_time_ns

    full = np.zeros((n_fine, out_dim), np.float32)
    for c in range(ncores):
        full[c * nsh:(c + 1) * nsh] = outs[c][:nsh]
    return full, exec_ns


def kernel(**inputs):
    out, _ = run(inputs)
    return out


# revision 17
# speedup vs baseline: 1.2541x; 1.0232x over previous
"""CFD-GCN Trainium2 kernel: 6-layer GCN + KNN-interpolate on 8 NeuronCores.

Strategy (node sharding, feature-major residency, v2):
  - Fine nodes are sharded 6250/core (padded to 6272 = 49*128).
  - Each GCN layer l: z = h @ W_l (dense, per-core shard, PE),
    AllGather(z) -> z_full (fp16), per-edge gather of source rows
    (grouped indirect DMA over ~3-4 dest tiles per call), scatter-add
    via one-hot matmuls into PSUM per 128-dest tile, bias + (relu) on
    ScalarE -> next h (kept feature-major in SBUF, which makes the next
    dense matmul transpose-free).
  - First layer runs sparse-first on the 6-wide input ((A h0) W0), last
    layer dense-first on the 3-wide output (A (h W5)), so only 4 of the
    6 layers move 512-wide data through the AllGather+gather path.
  - The one-hot scatter matrices S[e, d] = norm_e * (col_e == d) are
    built on the HOST (fp16) and streamed from DRAM, so DVE does no
    S-building work. Edges are sorted by destination; each 128-dest
    tile gets a data-dependent number of 128-edge blocks (max over
    cores so the program is SPMD-uniform), padded with (idx=0, norm=0)
    slots. Blocks of several tiles are gathered in one dma_gather call.
  - Everything on the wide path (z_full, gathered messages, S, h, W) is
    fp16; PSUM accumulation stays fp32.
  - KNN-interpolate: -d2 = 2 f.c - |c|^2 - |f|^2 via a K=3 matmul
    against all coarse nodes, DVE max8/max_index for the top-3,
    inverse-d2 weights, tiny indirect gather of coarse_y rows.
"""

import math
import numpy as np
import ml_dtypes

HDT = np.float16

# ---------------------------------------------------------------- constants
N_FINE = 50000
N_COARSE = 2000
HID = 512
OUT = 3
NCORES = 8
P = 128
GMAXB = 28  # max edge-blocks (128 edges each) per gather group

_PROGRAM_CACHE = {}


# ---------------------------------------------------------------- host side
def _wrap16(flat, P=128):
    """int16 index list -> dma_gather layout [P, len/16] (wrapped in 16
    partitions, replicated across the 8 Q7 cores)."""
    L = len(flat) // 16
    w = np.asarray(flat, np.int16).reshape(L, 16).T  # [16, L]
    return np.tile(w, (P // 16, 1))


class _Group:
    __slots__ = ("tiles", "blkA", "blkB", "blk", "gb0")


class _Plan:
    __slots__ = ("nt", "padsh", "half", "groups", "totblk", "gmax",
                 "blkA", "blkB", "key")


def _make_plan(cntA, cntB, nt, padsh, half):
    """Uniform (max-over-cores) per-tile block counts and gather groups."""
    blkA = np.ceil(np.maximum(cntA.max(axis=0), 1) / P).astype(int)
    blkB = np.ceil(np.maximum(cntB.max(axis=0), 1) / P).astype(int)

    groups = []
    cur = []
    for t in range(nt):
        bt = int(blkA[t] + blkB[t])
        curblk = sum(int(blkA[u] + blkB[u]) for u in cur)
        if cur and curblk + bt > GMAXB:
            groups.append(list(cur))
            cur = []
        cur.append(t)
    if cur:
        groups.append(cur)

    plan = _Plan()
    plan.nt, plan.padsh, plan.half = nt, padsh, half
    plan.blkA, plan.blkB = blkA, blkB
    plan.groups = []
    gb0 = 0
    for tiles in groups:
        g = _Group()
        g.tiles = []
        aoff = 0
        for t in tiles:
            g.tiles.append((t, aoff, int(blkA[t]), None, int(blkB[t])))
            aoff += int(blkA[t])
        g.blkA = aoff
        boff = 0
        tl = []
        for (t, ao, na, _, nb) in g.tiles:
            tl.append((t, ao, na, boff, nb))
            boff += nb
        g.tiles = tl
        g.blkB = boff
        g.blk = g.blkA + g.blkB
        g.gb0 = gb0
        gb0 += g.blk
        plan.groups.append(g)
    plan.totblk = gb0
    plan.gmax = max(g.blk for g in plan.groups)
    plan.key = (nt, padsh, half, tuple(int(x) for x in blkA),
                tuple(int(x) for x in blkB))
    return plan


def _preprocess_edges(edge_index, n_fine, ncores):
    """Sort edges by destination, shard by dest core, tile dests by 128.

    Edges of each (core, dest-tile) are split by source half (table rows
    0..HALF-1 vs HALF..) so indices fit int16; each (tile, half) segment
    is padded to a multiple of 128 edges with (idx=0, norm=0) slots.

    Returns (plan, per-core dict arrays {idx16, S}).
    """
    nsh = n_fine // ncores
    nt = math.ceil(nsh / P)
    padsh = nt * P
    half = (ncores // 2) * padsh

    row = np.asarray(edge_index[0]).astype(np.int64)
    col = np.asarray(edge_index[1]).astype(np.int64)
    loop = np.arange(n_fine, dtype=np.int64)
    row = np.concatenate([row, loop])
    col = np.concatenate([col, loop])

    deg = np.bincount(col, minlength=n_fine).astype(np.float32)
    dis = 1.0 / np.sqrt(deg)
    normv = (dis[row] * dis[col]).astype(np.float32)
    srcpad = ((row // nsh) * padsh + (row % nsh)).astype(np.int64)

    order = np.argsort(col, kind="stable")
    col_s, norm_s, srcpad_s = col[order], normv[order], srcpad[order]

    seg = {}
    cntA = np.zeros((ncores, nt), np.int64)
    cntB = np.zeros((ncores, nt), np.int64)
    for c in range(ncores):
        base = c * nsh
        for t in range(nt):
            lo, hi = base + t * P, min(base + (t + 1) * P, base + nsh)
            a = np.searchsorted(col_s, lo, "left")
            b = np.searchsorted(col_s, hi, "left")
            isa = srcpad_s[a:b] < half
            seg[(c, t)] = (a, b, isa)
            cntA[c, t] = int(isa.sum())
            cntB[c, t] = int((~isa).sum())

    plan = _make_plan(cntA, cntB, nt, padsh, half)

    # global block base per (tile, half)
    baseA = np.zeros(nt, np.int64)
    baseB = np.zeros(nt, np.int64)
    for g in plan.groups:
        for (t, ao, na, bo, nb) in g.tiles:
            baseA[t] = g.gb0 + ao
            baseB[t] = g.gb0 + g.blkA + bo

    out = []
    for c in range(ncores):
        idx_flat = np.zeros(plan.totblk * P, np.int64)
        pL, bL, cL, nL = [], [], [], []
        for t in range(nt):
            a, b, isa = seg[(c, t)]
            colrel = (col_s[a:b] - (c * nsh + t * P)).astype(np.int64)
            nrm = norm_s[a:b]
            sp = srcpad_s[a:b]
            for (mask, bbase, hoff) in ((isa, baseA[t], 0),
                                        (~isa, baseB[t], half)):
                src = sp[mask] - hoff
                nh = len(src)
                if nh == 0:
                    continue
                s = np.arange(nh)
                idx_flat[bbase * P + s] = src
                pL.append(s % P)
                bL.append(bbase + s // P)
                cL.append(colrel[mask])
                nL.append(nrm[mask])
        S_np = np.zeros((P, plan.totblk, P), np.float32)
        S_np[np.concatenate(pL), np.concatenate(bL),
             np.concatenate(cL)] = np.concatenate(nL)
        out.append({
            "idx16": _wrap16(idx_flat),
            "S": S_np.astype(HDT),
        })
    return plan, out


def _pad_shard(x, nsh, padsh, ncores):
    d = x.shape[1]
    out = np.zeros((ncores * padsh, d), x.dtype)
    for c in range(ncores):
        out[c * padsh: c * padsh + nsh] = x[c * nsh: (c + 1) * nsh]
    return out


# ---------------------------------------------------------------- device side
def build_program(n_fine, n_coarse, hid, out_dim, ncores, plan):
    import concourse.bass as bass
    import concourse.mybir as mybir
    from concourse.bacc import Bacc
    from concourse.tile import TileContext
    from concourse.masks import make_identity
    from contextlib import ExitStack

    F32 = mybir.dt.float32
    F16 = mybir.dt.float16
    I16 = mybir.dt.int16
    nt, padsh, half = plan.nt, plan.padsh, plan.half
    npad = ncores * padsh
    kc = hid // P
    totblk = plan.totblk
    gmax = plan.gmax
    rg = [list(range(ncores))]
    AF = mybir.ActivationFunctionType
    ALU = mybir.AluOpType
    IOO = bass.IndirectOffsetOnAxis
    ncpad = math.ceil(n_coarse / 512) * 512
    ncc = math.ceil(n_coarse / 512)

    nc = Bacc(num_devices=ncores, num_swdge_queues=2)

    # ---- kernel I/O (per core) ----
    h0 = nc.declare_dram_parameter("h0", [npad, P], F16, isOutput=False)
    idx16 = nc.declare_dram_parameter("idx16", [P, totblk * 8], I16,
                                      isOutput=False)
    S_dram = nc.declare_dram_parameter("S", [P, totblk, P], F16,
                                       isOutput=False)
    xposT = nc.declare_dram_parameter("xposT", [2, padsh], F32, isOutput=False)
    xpos_nm = nc.declare_dram_parameter("xpos_nm", [padsh, 2], F32,
                                        isOutput=False)
    coarseT = nc.declare_dram_parameter("coarseT", [2, n_coarse], F32,
                                        isOutput=False)
    ycoarse = nc.declare_dram_parameter("ycoarse", [n_coarse, out_dim], F32,
                                        isOutput=False)
    w_mid = [nc.declare_dram_parameter(n, [hid, hid], F16, isOutput=False)
             for n in ("w1", "w2", "we0", "we1")]
    b_mid = [nc.declare_dram_parameter(n, [hid], F32, isOutput=False)
             for n in ("b1", "b2", "be0", "be1")]
    w0 = nc.declare_dram_parameter("w0", [6, hid], F16, isOutput=False)
    b0 = nc.declare_dram_parameter("b0", [hid], F32, isOutput=False)
    wtop = nc.declare_dram_parameter("wtop", [out_dim, hid], F16,
                                     isOutput=False)
    w5 = nc.declare_dram_parameter("w5", [hid, out_dim], F16, isOutput=False)
    b5 = nc.declare_dram_parameter("b5", [out_dim], F32, isOutput=False)
    y_out = nc.declare_dram_parameter("out", [padsh, out_dim], F32,
                                      isOutput=True)

    # ---- internal DRAM ----
    zsh = [nc.dram_tensor(f"zsh{i}", [padsh, hid], F16) for i in range(4)]
    zfull = [nc.dram_tensor(f"zfull{i}", [npad, hid], F16, addr_space="Shared")
             for i in range(4)]
    z5sh = nc.dram_tensor("z5sh", [padsh, P], F16)
    z5full = nc.dram_tensor("z5full", [npad, P], F16, addr_space="Shared")

    with TileContext(nc) as tc:
        with ExitStack() as ctx:
            main = ctx.enter_context(tc.tile_pool(name="main", bufs=1))
            wpool = ctx.enter_context(tc.tile_pool(name="wpool", bufs=2))
            sp = ctx.enter_context(tc.tile_pool(name="sp", bufs=2))
            zp = ctx.enter_context(tc.tile_pool(name="zp", bufs=2))
            smallp = ctx.enter_context(tc.tile_pool(name="smallp", bufs=2))
            # PSUM: three pools, one shared tag each -> 6 banks max
            ppA = ctx.enter_context(tc.tile_pool(name="ppA", bufs=2,
                                                 space="PSUM"))
            ppB = ctx.enter_context(tc.tile_pool(name="ppB", bufs=2,
                                                 space="PSUM"))
            ppC = ctx.enter_context(tc.tile_pool(name="ppC", bufs=2,
                                                 space="PSUM"))

            def accps(shape):
                return ppA.tile(shape, F32, tag="acc", name="acc")

            def densps(shape):
                return ppB.tile(shape, F32, tag="dacc", name="dacc")

            def tps(shape):
                return ppC.tile(shape, F32, tag="tp", name="tp")

            # ---------- persistent tiles ----------
            hT = main.tile([P, kc, padsh], F16, tag="hT")
            y3n = main.tile([P, nt, out_dim], F32, tag="y3n")
            iden = main.tile([P, P], F32, tag="iden")
            idx_sb = main.tile([P, totblk * 8], I16, tag="idx_sb")
            wtop_sb = main.tile([out_dim, hid], F16, tag="wtop_sb")
            w0_sb = main.tile([6, hid], F16, tag="w0_sb")

            nc.sync.dma_start(out=idx_sb[:], in_=idx16[:, :])
            nc.sync.dma_start(out=wtop_sb[:], in_=wtop[:, :])
            nc.sync.dma_start(out=w0_sb[:], in_=w0[:, :])
            make_identity(nc, iden[:])

            # ---------- helpers ----------
            def load_w_mid(wd):
                # SWDGE casts f32 -> fp16 during the DMA
                w_sb = wpool.tile([P, kc, hid], F16, tag="w_sb")
                nc.sync.dma_start(
                    out=w_sb[:], in_=wd[:, :].rearrange("(k p) h -> p k h",
                                                        p=P))
                return w_sb

            def load_b_mid(bd):
                b_sb = wpool.tile([P, kc], F32, tag="b_sb")
                nc.sync.dma_start(out=b_sb[:],
                                  in_=bd[:].rearrange("(k p) -> p k", p=P))
                return b_sb

            # Two gather-DMA semaphores, alternating per gather group:
            # the pool-rotation WAR edges separate same-parity groups, so
            # each per-parity cumulative wait value is a valid observation
            # point for the race checker.
            gsems = [nc.alloc_semaphore(f"gather_dma{i}") for i in range(2)]
            for s in gsems:
                nc.gpsimd.sem_clear(s)
            gcnt = [0, 0]
            gpar = [0]

            def sparse_pass(tableA, tableB, elem, msg_tag, per_tile):
                """Grouped async gathers + per-tile one-hot matmuls.

                Gathers use PREPARE_ONLY + trigger_dma so the Pool engine
                only pays descriptor generation; the transfer overlaps
                compute, with Tile deferring the data deps to the DMA
                completion tick.

                per_tile(t, msg, S_sb, blocks) consumes the gathered
                messages; blocks is the list of local block indices of
                tile t within the group buffers.
                """
                for g in plan.groups:
                    msg = sp.tile([P, gmax, elem], F16, tag=msg_tag,
                                  name=msg_tag)
                    S_sb = sp.tile([P, gmax, P], F16, tag="S_" + msg_tag,
                                   name="S_sb")
                    par = gpar[0] % 2
                    gpar[0] += 1
                    nprep = 0
                    if g.blkA:
                        nc.gpsimd.dma_gather(
                            msg[:, 0:g.blkA, :], tableA,
                            idx_sb[:, g.gb0 * 8:(g.gb0 + g.blkA) * 8],
                            g.blkA * P, g.blkA * P, elem,
                            single_packet=False, prepare_only=True,
                            sem=gsems[par], queue_num=1)
                        nprep += 1
                    if g.blkB:
                        nc.gpsimd.dma_gather(
                            msg[:, g.blkA:g.blk, :], tableB,
                            idx_sb[:, (g.gb0 + g.blkA) * 8:
                                   (g.gb0 + g.blk) * 8],
                            g.blkB * P, g.blkB * P, elem,
                            single_packet=False, prepare_only=True,
                            sem=gsems[par], queue_num=1)
                        nprep += 1
                    if nprep:
                        nc.gpsimd.trigger_dma(count=None, queue_num=1)
                        gcnt[par] += 16 * nprep
                        canary = nc.vector.tensor_copy(
                            out=msg[0:1, 0:g.blk, 0:1],
                            in_=msg[0:1, 0:g.blk, 0:1])
                        canary._wait_ge(gsems[par], gcnt[par])
                    nc.sync.dma_start(out=S_sb[:, 0:g.blk, :],
                                      in_=S_dram[:, g.gb0:g.gb0 + g.blk, :])
                    for (t, ao, na, bo, nb) in g.tiles:
                        blocks = (list(range(ao, ao + na)) +
                                  list(range(g.blkA + bo, g.blkA + bo + nb)))
                        per_tile(t, msg, S_sb, blocks)

            # ---------- KNN (independent; writes y3n) ----------
            # -d2[m, n] = 2 f_m . c_n - |c_n|^2 - |f_m|^2 :
            #   matmul K=3 with lhsT rows [2fx, 2fy, -1], rhs rows
            #   [cx, cy, |c|^2], then a per-partition add of -|f_m|^2.
            # The prep runs up front; per-tile work is emitted in chunks
            # just before the first two AllGathers so PE/DVE chew on it
            # during the collectives.
            if True:
                kp = ctx.enter_context(tc.tile_pool(name="knn", bufs=2))
                mones_sb = kp.tile([1, P], F32, tag="mones_sb", bufs=1)
                nc.vector.memset(mones_sb[:], -1.0)
                coarse3 = kp.tile([3, n_coarse], F32, tag="coarse3", bufs=1)
                with tc.tile_pool(name="knnprep", bufs=1) as kprep:
                    nc.sync.dma_start(out=coarse3[0:2, :], in_=coarseT[:, :])
                    pones = kprep.tile([2, 1], F32, tag="pones")
                    nc.vector.memset(pones[:], 1.0)
                    csq = kprep.tile([1, n_coarse], F32, tag="csq")
                    for i in range(ncc):
                        a, b = i * 512, min((i + 1) * 512, n_coarse)
                        sqc = kprep.tile([2, 512], F32, tag="sqc")
                        nc.vector.tensor_tensor(out=sqc[:, : b - a],
                                                in0=coarse3[0:2, a:b],
                                                in1=coarse3[0:2, a:b],
                                                op=ALU.mult)
                        ps = tps([P, 512])
                        nc.tensor.matmul(out=ps[0:1, : b - a], lhsT=pones[:],
                                         rhs=sqc[:, : b - a], start=True,
                                         stop=True)
                        nc.vector.tensor_copy(out=csq[:, a:b],
                                              in_=ps[0:1, : b - a])
                    # row 2 (|c|^2) via DMA (compute engines can't start at
                    # partition 2)
                    nc.sync.dma_start(out=coarse3[2:3, :], in_=csq[:])

                    # -|f|^2 per node, node-major: [P, nt]
                    xnm = kprep.tile([P, nt, 2], F32, tag="xnm")
                    nc.sync.dma_start(
                        out=xnm[:],
                        in_=xpos_nm[:, :].rearrange("(t p) d -> p t d", p=P))
                    sqn = kprep.tile([P, nt, 2], F32, tag="sqn")
                    nc.vector.tensor_tensor(out=sqn[:], in0=xnm[:], in1=xnm[:],
                                            op=ALU.mult)
                    fsqneg = kp.tile([P, nt], F32, tag="fsqneg", bufs=1)
                    nc.vector.tensor_reduce(out=fsqneg[:], in_=sqn[:],
                                            axis=mybir.AxisListType.X,
                                            op=ALU.add, negate=True)

                def knn_tile(t):
                    tp = t * P
                    xp_t = kp.tile([2, P], F32, tag="xp_t")
                    nc.sync.dma_start(out=xp_t[:], in_=xposT[:, tp:tp + P])
                    lhsT3 = kp.tile([3, P], F32, tag="lhsT3")
                    nc.vector.tensor_scalar_mul(lhsT3[0:2, :], xp_t[:], 2.0)
                    nc.sync.dma_start(out=lhsT3[2:3, :], in_=mones_sb[:])

                    d2 = kp.tile([P, ncpad], F32, tag="d2", bufs=1)
                    for i in range(ncc):
                        a, b = i * 512, min((i + 1) * 512, n_coarse)
                        dps = densps([P, 512])
                        nc.tensor.matmul(out=dps[:, : b - a], lhsT=lhsT3[:],
                                         rhs=coarse3[:, a:b], start=True,
                                         stop=True)
                        nc.vector.tensor_scalar(out=d2[:, a:b],
                                                in0=dps[:, : b - a],
                                                scalar1=fsqneg[:, t:t + 1],
                                                scalar2=None, op0=ALU.add)
                    vals = kp.tile([P, 8], F32, tag="vals")
                    nc.vector.max(out=vals[:], in_=d2[:, 0:n_coarse])
                    idxs = kp.tile([P, 8], mybir.dt.uint32, tag="idxs")
                    nc.vector.max_index(out=idxs[:], in_max=vals[:],
                                        in_values=d2[:, 0:n_coarse])
                    wv = kp.tile([P, 3], F32, tag="wv")
                    nc.vector.tensor_scalar(out=wv[:], in0=vals[:, 0:3],
                                            scalar1=-1.0, scalar2=1e-16,
                                            op0=ALU.mult, op1=ALU.max)
                    nc.vector.reciprocal(out=wv[:], in_=wv[:])
                    wsum = kp.tile([P, 1], F32, tag="wsum")
                    nc.vector.tensor_reduce(out=wsum[:], in_=wv[:],
                                            axis=mybir.AxisListType.X,
                                            op=ALU.add)
                    nc.vector.reciprocal(out=wsum[:], in_=wsum[:])
                    nc.vector.tensor_scalar(out=wv[:], in0=wv[:],
                                            scalar1=wsum[:, 0:1], scalar2=None,
                                            op0=ALU.mult)
                    yg = kp.tile([P, 3, out_dim], F32, tag="yg")
                    for k3 in range(3):
                        nc.gpsimd.indirect_dma_start(
                            out=yg[:, k3, :], out_offset=None,
                            in_=ycoarse[:, :],
                            in_offset=IOO(ap=idxs[:, k3:k3 + 1], axis=0))
                    tmp = kp.tile([P, out_dim], F32, tag="tmp")
                    nc.vector.tensor_scalar(out=y3n[:, t, :], in0=yg[:, 0, :],
                                            scalar1=wv[:, 0:1], scalar2=None,
                                            op0=ALU.mult)
                    for k in (1, 2):
                        nc.vector.tensor_scalar(out=tmp[:], in0=yg[:, k, :],
                                                scalar1=wv[:, k:k + 1],
                                                scalar2=None, op0=ALU.mult)
                        nc.vector.tensor_tensor(out=y3n[:, t, :],
                                                in0=y3n[:, t, :],
                                                in1=tmp[:], op=ALU.add)

            # ---------- pre0: q = A h0 (6-wide), then z0T = W0^T q, relu ----
            b0_sb = load_b_mid(b0)

            def pre0_tile(t, msg, S_sb, blocks):
                tp = t * P
                q = accps([P, max(P, kc * P)])
                nbk = len(blocks)
                for i, bj in enumerate(blocks):
                    nc.tensor.matmul(out=q[0:6, 0:P], lhsT=msg[:, bj, 0:6],
                                     rhs=S_sb[:, bj, :], start=(i == 0),
                                     stop=(i == nbk - 1))
                q_sb = smallp.tile([6, P], F16, tag="q_sb")
                nc.vector.tensor_copy(out=q_sb[:], in_=q[0:6, 0:P])
                for jj in range(kc):
                    z0 = densps([P, hid])
                    nc.tensor.matmul(out=z0[:, 0:P],
                                     lhsT=w0_sb[:, jj * P:(jj + 1) * P],
                                     rhs=q_sb[:], start=True, stop=True)
                    nc.scalar.activation(out=hT[:, jj, tp:tp + P],
                                         in_=z0[:, 0:P], func=AF.Relu,
                                         bias=b0_sb[:, jj:jj + 1])

            sparse_pass(h0[0:half, :], h0[half:, :], P, "msgS", pre0_tile)

            # ---------- middle layers ----------
            def dense_mid(w_sb, zsh_d, li):
                for t in range(nt):
                    tp = t * P
                    zps = densps([P, hid])
                    for k in range(kc):
                        nc.tensor.matmul(out=zps[:], lhsT=hT[:, k, tp:tp + P],
                                         rhs=w_sb[:, k, :], start=(k == 0),
                                         stop=(k == kc - 1) and li != 2)
                    if li == 2:
                        pt3 = tps([P, P])
                        nc.tensor.transpose(out=pt3[0:out_dim, 0:P],
                                            in_=y3n[:, t, :], identity=iden[:])
                        y3t_T = smallp.tile([out_dim, P], F16, tag="y3t_T")
                        nc.vector.tensor_copy(out=y3t_T[:],
                                              in_=pt3[0:out_dim, 0:P])
                        nc.tensor.matmul(out=zps[:], lhsT=y3t_T[:],
                                         rhs=wtop_sb[:, :], start=False,
                                         stop=True)
                    z_sb = zp.tile([P, hid], F16, tag="z_sb")
                    nc.scalar.activation(out=z_sb[:], in_=zps[:], func=AF.Copy)
                    nc.sync.dma_start(out=zsh_d[tp:tp + P, :], in_=z_sb[:])

            def mid_tile_fn(b_sb):
                def mid_tile(t, msg, S_sb, blocks):
                    tp = t * P
                    hps = accps([P, kc * P])
                    nbk = len(blocks)
                    for cc in range(kc):
                        for i, bj in enumerate(blocks):
                            nc.tensor.matmul(
                                out=hps[:, cc * P:(cc + 1) * P],
                                lhsT=msg[:, bj, cc * P:(cc + 1) * P],
                                rhs=S_sb[:, bj, :], start=(i == 0),
                                stop=(i == nbk - 1))
                    for cc in range(kc):
                        nc.scalar.activation(out=hT[:, cc, tp:tp + P],
                                             in_=hps[:, cc * P:(cc + 1) * P],
                                             func=AF.Relu,
                                             bias=b_sb[:, cc:cc + 1])
                return mid_tile

            knn_chunks = [range(0, (nt + 1) // 2), range((nt + 1) // 2, nt),
                          range(0), range(0)]
            for li in range(4):
                w_sb = load_w_mid(w_mid[li])
                b_sb = load_b_mid(b_mid[li])
                dense_mid(w_sb, zsh[li], li)
                for t in knn_chunks[li]:
                    knn_tile(t)
                nc.gpsimd.collective_compute(
                    "AllGather", ALU.bypass, replica_groups=rg,
                    ins=[zsh[li][:, :]], outs=[zfull[li][:, :]])
                sparse_pass(zfull[li][0:half, :], zfull[li][half:, :], hid,
                            "msgW", mid_tile_fn(b_sb))

            # ---------- end2: z5T = W5^T h, transpose, AG, sparse3 + bias ----
            w5_sb = main.tile([P, kc, out_dim], F16, tag="w5_sb")
            nc.sync.dma_start(out=w5_sb[:],
                              in_=w5[:, :].rearrange("(k p) o -> p k o",
                                                     p=P))
            b5_sb = main.tile([out_dim, 1], F32, tag="b5_sb")
            nc.sync.dma_start(out=b5_sb[:], in_=b5[:, None])

            z5w_pp = [main.tile([P, P], F16, tag=f"z5wpp{i}", name="z5wpp")
                      for i in range(2)]
            for m in z5w_pp:
                nc.vector.memset(m[:], 0.0)

            for t in range(nt):
                tp = t * P
                z5ps = densps([P, hid])
                for k in range(kc):
                    nc.tensor.matmul(out=z5ps[0:out_dim, 0:P],
                                     lhsT=w5_sb[:, k, :],
                                     rhs=hT[:, k, tp:tp + P], start=(k == 0),
                                     stop=(k == kc - 1))
                # transpose [3, 128] -> [128, 3] and store node-major
                z5T_sb = smallp.tile([out_dim, P], F32, tag="z5T_sb")
                nc.vector.tensor_copy(out=z5T_sb[:], in_=z5ps[0:out_dim, 0:P])
                ptp = tps([P, P])
                nc.tensor.transpose(out=ptp[:, 0:out_dim], in_=z5T_sb[:],
                                    identity=iden[0:out_dim, 0:out_dim])
                z5_sb = z5w_pp[t % 2]
                nc.vector.tensor_copy(out=z5_sb[:, 0:out_dim],
                                      in_=ptp[:, 0:out_dim])
                nc.sync.dma_start(out=z5sh[tp:tp + P, :], in_=z5_sb[:])
            nc.gpsimd.collective_compute(
                "AllGather", ALU.bypass, replica_groups=rg,
                ins=[z5sh[:, :]], outs=[z5full[:, :]])

            def end_tile(t, msg, S_sb, blocks):
                tp = t * P
                ops = accps([P, max(P, kc * P)])
                nbk = len(blocks)
                for i, bj in enumerate(blocks):
                    nc.tensor.matmul(out=ops[0:out_dim, 0:P],
                                     lhsT=msg[:, bj, 0:out_dim],
                                     rhs=S_sb[:, bj, :], start=(i == 0),
                                     stop=(i == nbk - 1))
                oT = smallp.tile([out_dim, P], F32, tag="oT")
                nc.vector.tensor_scalar(out=oT[:], in0=ops[0:out_dim, 0:P],
                                        scalar1=b5_sb[:, 0:1], scalar2=None,
                                        op0=ALU.add)
                po = tps([P, P])
                nc.tensor.transpose(out=po[:, 0:out_dim], in_=oT[:],
                                    identity=iden[0:out_dim, 0:out_dim])
                o_sb = smallp.tile([P, out_dim], F32, tag="o_sb")
                nc.vector.tensor_copy(out=o_sb[:], in_=po[:, 0:out_dim])
                nc.sync.dma_start(out=y_out[tp:tp + P, :], in_=o_sb[:])

            sparse_pass(z5full[0:half, :], z5full[half:, :], P, "msgS",
                        end_tile)

    nc.finalize()
    return nc


# ---------------------------------------------------------------- entry point
def _prepare(inputs, n_fine, n_coarse, hid, out_dim, ncores):
    x = np.asarray(inputs["x"], np.float32)
    sdf = np.asarray(inputs["sdf"], np.float32)
    coarse_x = np.asarray(inputs["coarse_x"], np.float32)
    coarse_y = np.asarray(inputs["coarse_y"], np.float32)
    edge_index = np.asarray(inputs["edge_index"])

    plan, edges = _preprocess_edges(edge_index, n_fine, ncores)
    nsh = n_fine // ncores
    padsh = plan.padsh

    h0 = np.zeros((n_fine, P), np.float32)
    h0[:, 0:5] = x
    h0[:, 5:6] = sdf
    h0pad = _pad_shard(h0, nsh, padsh, ncores).astype(HDT)

    xpos = x[:, :2].astype(np.float32)
    xposT = []
    xpos_nm_l = []
    for c in range(ncores):
        xx = np.zeros((2, padsh), np.float32)
        xx[:, :nsh] = xpos[c * nsh:(c + 1) * nsh].T
        xposT.append(xx)
        xn = np.zeros((padsh, 2), np.float32)
        xn[:nsh] = xpos[c * nsh:(c + 1) * nsh]
        xpos_nm_l.append(xn)
    coarseT = np.ascontiguousarray(coarse_x[:, :2].T).astype(np.float32)

    in_maps = []
    for c in range(ncores):
        m = {
            "h0": h0pad,
            "idx16": edges[c]["idx16"],
            "S": edges[c]["S"],
            "xposT": xposT[c], "xpos_nm": xpos_nm_l[c],
            "coarseT": coarseT, "ycoarse": coarse_y,
            "w0": np.asarray(inputs["pre_W0"], np.float32).astype(HDT),
            "b0": np.asarray(inputs["pre_b0"], np.float32),
            "w1": np.asarray(inputs["pre_W1"], np.float32).astype(HDT),
            "b1": np.asarray(inputs["pre_b1"], np.float32),
            "w2": np.asarray(inputs["pre_W2"], np.float32).astype(HDT),
            "b2": np.asarray(inputs["pre_b2"], np.float32),
            # end_W0 is [OUT+HID, HID]: top 3 rows couple y3, rest couple h
            "wtop": np.ascontiguousarray(
                np.asarray(inputs["end_W0"], np.float32)[:out_dim]).astype(HDT),
            "we0": np.ascontiguousarray(
                np.asarray(inputs["end_W0"], np.float32)[out_dim:]).astype(HDT),
            "be0": np.asarray(inputs["end_b0"], np.float32),
            "we1": np.asarray(inputs["end_W1"], np.float32).astype(HDT),
            "be1": np.asarray(inputs["end_b1"], np.float32),
            "w5": np.asarray(inputs["end_W2"], np.float32).astype(HDT),
            "b5": np.asarray(inputs["end_b2"], np.float32),
        }
        in_maps.append(m)
    return plan, in_maps


def run(inputs, n_fine=N_FINE, n_coarse=N_COARSE, hid=HID, out_dim=OUT,
        ncores=NCORES, sim=False, trace=False):
    plan, in_maps = _prepare(inputs, n_fine, n_coarse, hid, out_dim, ncores)
    key = (n_fine, n_coarse, hid, out_dim, ncores) + plan.key
    if key not in _PROGRAM_CACHE:
        _PROGRAM_CACHE[key] = build_program(n_fine, n_coarse, hid, out_dim,
                                            ncores, plan)
    nc = _PROGRAM_CACHE[key]

    nsh = n_fine // ncores
    if sim:
        from concourse.bass_interp import MultiCoreSim
        ms = MultiCoreSim(nc, ncores, num_workers=1)
        for c in range(ncores):
            for k, v in in_maps[c].items():
                ms.cores[c].tensor(k)[:] = v
        ms.simulate()
        outs = [np.array(ms.cores[c].tensor("out")) for c in range(ncores)]
        exec_ns = None
    else:
        from concourse.bass_utils import run_bass_kernel_spmd
        res = run_bass_kernel_spmd(nc, in_maps, list(range(ncores)),
                                   trace=trace)
        outs = [res.results[c]["out"] for c in range(ncores)]
        exec_ns = res.exec_time_ns

    full = np.zeros((n_fine, out_dim), np.float32)
    for c in range(ncores):
        full[c * nsh:(c + 1) * nsh] = outs[c][:nsh]
    return full, exec_ns


def kernel(**inputs):
    out, _ = run(inputs)
    return out
